# revision 19
# baseline (speedup 1.0000x reference)
"""BertLinearSelfAttention on 8 Trainium2 NeuronCores.

Problem (per reference):
  q = hs @ Wq.T + bq ; k = hs @ Wk.T + bk ; v = hs @ Wv.T + bv   (B,S,D)
  per head: scores = q @ k.T ; probs = scores * (mask >= 0) ; ctx = probs @ v
  B=2, S=2048, D=1024, H=16, HD=64. No softmax, binary key mask.

Key algebraic move: with no softmax, the attention is associative:
  ctx_h = (q_h k_h^T * mask) v_h = q_h @ M_h,   M_h = k_h^T diag(mask) v_h
M_h is only [64, 64] per head, so the O(S^2) scores/probs work disappears.
Remaining FLOPs are the q/k/v projections plus tiny M and M-apply matmuls.
Masked keys contribute nothing, so k/v are computed over host-compacted
valid keys only (CAP slots, zero-padded; full-width fallback otherwise).

Sharding: core c = 4*b + g handles batch b and head group g (4 heads,
256 output features). Pure SPMD, no collectives; host gathers the output.

Per-core device program (all matmul operands fp16; fp32 PSUM accumulation):
  phase 1 (kv stream is only 3.25 MiB, so it goes first while the larger
  q stream arrives behind it on the same HWDGE ring):
    kv [CAP, 512]  = xkv [Wk_g^T|Wv_g^T]  (xkvt: host chunk-transposed)
    M2_p [128,128] = kv_k(pair p)^T @ kv_v(pair p), interleaved one key
                     chunk behind the kv matmuls so they ride the same
                     PE stream (diag 64x64 blocks are M_h; cross terms
                     are zeroed when copied to SBUF)
  phase 2, per 512-query strip (xt is sbk-major so each strip only
  needs its own piece; output DMAs spread across the phase):
    qT strip = Wq_g hs^T ; ctxT strip = blockdiag(M)^T @ qT -> OUT fp16
A few data-independent warmup matmuls run during the DMA head to flip
the PE HAM clock gate to 8/8 before the real work arrives.

Measured end-to-end rel err ~7e-4.
"""
import numpy as np
import concourse.mybir as mybir
import concourse.tile as tile
from concourse import bacc
from concourse.bass_utils import run_bass_kernel_spmd

f32 = mybir.dt.float32
f32r = mybir.dt.float32r
fp16 = mybir.dt.float16

B = 2
S = 2048
D = 1024
DL = 256          # output features per core (4 heads x 64)
KC = D // 128     # 8 contraction chunks
MC = DL // 128    # 2 feature chunks (head pairs)
NSB = S // 512    # 4 query strips
N_CORES = 8
CAP = 1152        # compacted key slots (valid ~Binom(2048,.5): mean 1024,
                  # sd 22.6; 1152 is ~5.7 sigma; fallback covers more)

_cache = {}


def _build(compact, with_bias):
    skv = (CAP if compact else S) // 128   # key chunks
    use_kvm = (not compact) or with_bias   # per-key scale on kv drains
    nc = bacc.Bacc("TRN2", target_bir_lowering=False, debug=False,
                   num_devices=N_CORES)
    XT = nc.declare_dram_parameter("xt", [128, KC * S], fp16, isOutput=False)
    XKVT = nc.declare_dram_parameter("xkvt", [128, skv * D], fp16,
                                     isOutput=False)
    WQT = nc.declare_dram_parameter("wqt", [128, KC * DL], fp16,
                                    isOutput=False)
    WKV = nc.declare_dram_parameter("wkv", [128, KC * 2 * DL], fp16,
                                    isOutput=False)
    if with_bias:
        BQ2 = nc.declare_dram_parameter("bq2", [128, MC], f32, isOutput=False)
        BKV = nc.declare_dram_parameter("bkv", [1, 2 * DL], fp16,
                                        isOutput=False)
        ONE = nc.declare_dram_parameter("ones1", [1, 128], fp16,
                                        isOutput=False)
    if use_kvm:
        KVM = nc.declare_dram_parameter("kvm2", [128, skv], f32,
                                        isOutput=False)
    OUT = nc.declare_dram_parameter("out", [DL, S], fp16, isOutput=True)

    with tile.TileContext(nc) as tc:
        with tc.tile_pool(name="sb", bufs=1) as sb, \
             tc.tile_pool(name="stg", bufs=4) as stg:

            wq_sb = sb.tile([128, KC * DL], fp16, tag="wq")
            xt_sb = sb.tile([128, KC * S], fp16, tag="xt")
            wkv_sb = sb.tile([128, KC * 2 * DL], fp16, tag="wkv")
            xkvt_sb = sb.tile([128, skv * D], fp16, tag="xkvt")
            qT = [sb.tile([128, S], fp16, tag=f"qT{p}", name=f"qT{p}")
                  for p in range(MC)]
            kv_sb = sb.tile([128, skv * 2 * DL], fp16, tag="kv")
            m2s = [sb.tile([128, 128], fp16, tag=f"m2s{p}", name=f"m2s{p}")
                   for p in range(MC)]
            if with_bias:
                bq2 = sb.tile([128, MC], f32, tag="bq2")
                bkv = sb.tile([1, 2 * DL], fp16, tag="bkv")
                ones1 = sb.tile([1, 128], fp16, tag="ones1")
            if use_kvm:
                kvm = sb.tile([128, skv], f32, tag="kvm")

            # DMA order matches PE consumption: kv stream first (smaller,
            # feeds the first phase), then the q stream which has the whole
            # kv phase to arrive. Single HWDGE ring -> FIFO at full BW.
            if with_bias:
                nc.sync.dma_start(bq2[:], BQ2[:, :])
                nc.sync.dma_start(bkv[:], BKV[:, :])
                nc.sync.dma_start(ones1[:], ONE[:, :])
            if use_kvm:
                nc.sync.dma_start(kvm[:], KVM[:, :])
            nc.sync.dma_start(wkv_sb[:], WKV[:, :])
            cuts = [0, D, 3 * D, 6 * D, skv * D]
            for a, b in zip(cuts, cuts[1:]):
                nc.sync.dma_start(xkvt_sb[:, a:b], XKVT[:, a:b])
            nc.sync.dma_start(wq_sb[:], WQT[:, :])
            for i in range(2):
                nc.sync.dma_start(xt_sb[:, i * 8192:(i + 1) * 8192],
                                  XT[:, i * 8192:(i + 1) * 8192])

            # PE warmup: data-independent matmuls on an uninitialized tile
            # run during the DMA head; ~3.5us of activity flips the HAM
            # clock gate to 8/8 so the real matmuls start at 2.4 GHz.
            junk = sb.tile([128, 512], fp16, tag="junk")
            nc.gpsimd.memset(junk[:], 0.0)
            with tc.tile_pool(name="psw", bufs=1, space="PSUM") as psw:
                wps = psw.tile([128, 512], f32, tag="warm")
                for _ in range(12):
                    nc.tensor.matmul(wps[:], junk[:, 0:128], junk[:],
                                     start=True, stop=True)

            # ---- K/V projection over (compacted) keys, with the M-pair
            # matmuls (M2_p = k_p^T v_p) interleaved one chunk behind so
            # they ride the kv matmul stream instead of a separate phase.
            # The psm pool stays open into phase 2: the last M matmuls and
            # the m2s copies slot between the first q groups, so the PE
            # never waits on the final kv drain.
            with tc.tile_pool(name="psm", bufs=1, space="PSUM") as psm, \
                 tc.tile_pool(name="psq", bufs=2, space="PSUM") as psq:
                mps = [psm.tile([128, 128], f32, tag=f"m2_{p}",
                                name=f"m2_{p}") for p in range(MC)]
                # claim a fresh PSUM bank for the first q group NOW, so it
                # isn't handed a just-freed kv bank whose drain is still in
                # flight at the phase transition
                qp_first = psq.tile([128, 512], f32, tag="qp", name="qp_first")

                def m_mms(j):
                    for p in range(MC):
                        nc.tensor.matmul(
                            mps[p][:],
                            kv_sb[:, j * 512 + p * 128:j * 512 + (p + 1) * 128],
                            kv_sb[:, j * 512 + 256 + p * 128:
                                  j * 512 + 256 + (p + 1) * 128],
                            start=(j == 0), stop=(j == skv - 1))

                with tc.tile_pool(name="psk", bufs=3, space="PSUM") as psk:
                    for j in range(skv):
                        pv = psk.tile([128, 512], f32, tag="kv")
                        if with_bias:
                            nc.tensor.matmul(pv[:], ones1[:], bkv[:],
                                             start=True, stop=False)
                        for kc in range(KC):
                            nc.tensor.matmul(
                                pv[:],
                                xkvt_sb[:, j * D + kc * 128:
                                        j * D + (kc + 1) * 128],
                                wkv_sb[:, kc * 512:(kc + 1) * 512],
                                start=(kc == 0 and not with_bias),
                                stop=(kc == KC - 1))
                        if use_kvm:
                            nc.vector.tensor_scalar_mul(
                                kv_sb[:, j * 512:(j + 1) * 512], pv[:],
                                kvm[:, j:j + 1])
                        else:
                            nc.vector.tensor_copy(
                                kv_sb[:, j * 512:(j + 1) * 512], pv[:])
                        if j > 0:
                            m_mms(j - 1)

                # ---- Q projection + M apply, strip by strip --------------
                # xt is laid out sbk-major on the host, so strip sbk only
                # needs its own 1 MiB piece; kc is the inner (accumulation)
                # loop. The ctx matmul and output DMA trail each strip,
                # spreading the output over the whole phase.
                with tc.tile_pool(name="psc", bufs=4, space="PSUM") as psc:

                    def q_group(sbk, mc, qp=None):
                        if qp is None:
                            qp = psq.tile([128, 512], f32, tag="qp")
                        for kc in range(KC):
                            nc.tensor.matmul(
                                qp[:],
                                wq_sb[:, kc * DL + mc * 128:
                                      kc * DL + mc * 128 + 128],
                                xt_sb[:, sbk * (KC * 512) + kc * 512:
                                      sbk * (KC * 512) + (kc + 1) * 512],
                                start=(kc == 0), stop=(kc == KC - 1))
                        o = sbk * 512
                        if with_bias:
                            if sbk == NSB - 1:
                                nc.vector.tensor_scalar_add(
                                    qT[mc][:, o:o + 256], qp[:, 0:256],
                                    bq2[:, mc:mc + 1])
                                nc.scalar.add(
                                    qT[mc][:, o + 256:o + 512],
                                    qp[:, 256:512], bq2[:, mc:mc + 1])
                            else:
                                nc.vector.tensor_scalar_add(
                                    qT[mc][:, o:o + 512], qp[:],
                                    bq2[:, mc:mc + 1])
                        elif sbk == NSB - 1:
                            # last strip: half-width drains run on DVE and
                            # ACT in parallel so the final ctx matmuls start
                            # ~0.35us sooner
                            nc.vector.tensor_copy(qT[mc][:, o:o + 256],
                                                  qp[:, 0:256])
                            nc.scalar.copy(qT[mc][:, o + 256:o + 512],
                                           qp[:, 256:512])
                        else:
                            nc.vector.tensor_copy(qT[mc][:, o:o + 512],
                                                  qp[:])

                    def ctx_out(sbk, p):
                        cp = psc.tile([128, 512], f32, tag="ctx")
                        nc.tensor.matmul(cp[:], m2s[p][:],
                                         qT[p][:, sbk * 512:(sbk + 1) * 512],
                                         start=True, stop=True)
                        st = stg.tile([128, 512], fp16, tag="st")
                        if sbk == NSB - 1:
                            # last strip: parallel half drains + two 64 KB
                            # output DMAs on the two idle HWDGE rings
                            nc.vector.tensor_copy(st[:, 0:256], cp[:, 0:256])
                            nc.scalar.copy(st[:, 256:512], cp[:, 256:512])
                            r0, r1 = (nc.scalar, nc.sync)
                            r0.dma_start(
                                OUT[p * 128:(p + 1) * 128,
                                    sbk * 512:sbk * 512 + 256], st[:, 0:256])
                            r1.dma_start(
                                OUT[p * 128:(p + 1) * 128,
                                    sbk * 512 + 256:(sbk + 1) * 512],
                                st[:, 256:512])
                        else:
                            nc.scalar.copy(st[:], cp[:])
                            out_eng = nc.scalar if p == 0 else nc.sync
                            out_eng.dma_start(
                                OUT[p * 128:(p + 1) * 128,
                                    sbk * 512:(sbk + 1) * 512], st[:])

                    for sbk in range(NSB):
                        for mc in range(MC):
                            q_group(sbk, mc,
                                    qp_first if sbk == 0 and mc == 0 else None)
                            if sbk == 0 and mc == 0:
                                # last M chunk + diag extraction ride here,
                                # hidden behind the first q group's drain
                                m_mms(skv - 1)
                                for p in range(MC):
                                    mp = mps[p]
                                    nc.vector.tensor_copy(
                                        m2s[p][0:64, 0:64], mp[0:64, 0:64])
                                    nc.scalar.copy(m2s[p][64:128, 64:128],
                                                   mp[64:128, 64:128])
                                    nc.vector.tensor_scalar_mul(
                                        m2s[p][0:64, 64:128],
                                        mp[0:64, 64:128], 0.0)
                                    nc.vector.tensor_scalar_mul(
                                        m2s[p][64:128, 0:64],
                                        mp[64:128, 0:64], 0.0)
                        for p in range(MC):
                            ctx_out(sbk, p)

    nc.compile()
    return nc


def _get_nc(compact, with_bias):
    key = (compact, with_bias)
    if key not in _cache:
        _cache[key] = _build(compact, with_bias)
    return _cache[key]


def _chunkT(a):
    """[R, D] row-major -> [128, (D//128)*R]: out[p, kc*R + r] = a[r, kc*128+p]."""
    R, Din = a.shape
    return np.ascontiguousarray(
        a.T.reshape(Din // 128, 128, R).transpose(1, 0, 2).reshape(128, -1))


def _make_in_maps(hidden_states, attention_mask, Wq, bq, Wk, bk, Wv, bv):
    hs16 = np.asarray(hidden_states, dtype=np.float32).astype(np.float16)
    am = np.asarray(attention_mask, dtype=np.float32)
    bq = np.asarray(bq, np.float32)
    bk = np.asarray(bk, np.float32)
    bv = np.asarray(bv, np.float32)
    Wq = np.asarray(Wq, np.float32)
    Wk = np.asarray(Wk, np.float32)
    Wv = np.asarray(Wv, np.float32)
    with_bias = bool(bq.any() or bk.any() or bv.any())

    compact = True
    xkvts, kvms = [], []
    for b in range(B):
        valid = np.nonzero(am[b, 0, 0, :] >= 0)[0]
        if len(valid) > CAP:
            compact = False
            break
        xkv = np.zeros((CAP, D), np.float16)
        xkv[:len(valid)] = hs16[b][valid]
        xkvts.append(xkv)
        kvmv = np.zeros(CAP, np.float32)
        kvmv[:len(valid)] = 1.0
        kvms.append(kvmv)

    skv = (CAP if compact else S) // 128
    use_kvm = (not compact) or with_bias

    # xt sbk-major: [p, sbk*4096 + kc*512 + s] = x[sbk*512+s, kc*128+p]
    xts = [np.ascontiguousarray(
        hs16[b].T.reshape(KC, 128, NSB, 512).transpose(1, 2, 0, 3)
        .reshape(128, KC * S)) for b in range(B)]
    if compact:
        xkvt_blk = []
        for b in range(B):
            xkv = xkvts[b]
            blocks = [_chunkT(xkv[j * 128:(j + 1) * 128]) for j in range(skv)]
            xkvt_blk.append(np.ascontiguousarray(np.concatenate(blocks, 1)))
    else:
        xkvt_blk = []
        for b in range(B):
            blocks = [_chunkT(hs16[b][j * 128:(j + 1) * 128])
                      for j in range(skv)]
            xkvt_blk.append(np.ascontiguousarray(np.concatenate(blocks, 1)))

    in_maps = []
    for c in range(N_CORES):
        b, g = divmod(c, 4)
        sl = slice(g * DL, (g + 1) * DL)
        wq_sel = Wq[sl].astype(np.float16)            # [256, 1024]
        wkv_sel = np.vstack([Wk[sl], Wv[sl]]).astype(np.float16)  # [512, 1024]
        m = {
            "xt": xts[b],
            "xkvt": xkvt_blk[b],
            # [p, kc*DL + f] = W[f, kc*128+p]
            "wqt": np.ascontiguousarray(
                wq_sel.T.reshape(KC, 128, DL).transpose(1, 0, 2)
                .reshape(128, KC * DL)),
            "wkv": np.ascontiguousarray(
                wkv_sel.T.reshape(KC, 128, 2 * DL).transpose(1, 0, 2)
                .reshape(128, KC * 2 * DL)),
        }
        if with_bias:
            m["bq2"] = np.ascontiguousarray(
                bq[sl].reshape(MC, 128).T.astype(np.float32))
            m["bkv"] = np.ascontiguousarray(
                np.concatenate([bk[sl], bv[sl]]).reshape(1, 2 * DL)
                .astype(np.float16))
            m["ones1"] = np.ones((1, 128), np.float16)
        if use_kvm:
            if compact:
                kvmv = kvms[b]
            else:
                kvmv = (am[b, 0, 0, :] >= 0).astype(np.float32)
            m["kvm2"] = np.ascontiguousarray(kvmv.reshape(skv, 128).T)
        in_maps.append(m)
    return (compact, with_bias), in_maps


def _gather(results):
    out = np.empty((B, S, D), np.float32)
    for c in range(N_CORES):
        b, g = divmod(c, 4)
        out[b, :, g * DL:(g + 1) * DL] = results[c]["out"].T.astype(np.float32)
    return out


def run_sharded(variant, in_maps, **kw):
    compact, with_bias = variant if isinstance(variant, tuple) else (variant, False)
    nc = _get_nc(compact, with_bias)
    return run_bass_kernel_spmd(nc, in_maps, core_ids=list(range(N_CORES)), **kw)


def kernel(hidden_states, attention_mask, Wq, bq, Wk, bk, Wv, bv):
    variant, in_maps = _make_in_maps(hidden_states, attention_mask,
                                     Wq, bq, Wk, bk, Wv, bv)
    res = run_sharded(variant, in_maps)
    return _gather(res.results)


# revision 20
# speedup vs baseline: 1.0037x; 1.0037x over previous
"""BertLinearSelfAttention on 8 Trainium2 NeuronCores.

Problem (per reference):
  q = hs @ Wq.T + bq ; k = hs @ Wk.T + bk ; v = hs @ Wv.T + bv   (B,S,D)
  per head: scores = q @ k.T ; probs = scores * (mask >= 0) ; ctx = probs @ v
  B=2, S=2048, D=1024, H=16, HD=64. No softmax, binary key mask.

Key algebraic move: with no softmax, the attention is associative:
  ctx_h = (q_h k_h^T * mask) v_h = q_h @ M_h,   M_h = k_h^T diag(mask) v_h
M_h is only [64, 64] per head, so the O(S^2) scores/probs work disappears.
Remaining FLOPs are the q/k/v projections plus tiny M and M-apply matmuls.
Masked keys contribute nothing, so k/v are computed over host-compacted
valid keys only (CAP slots, zero-padded; full-width fallback otherwise).

Sharding: core c = 4*b + g handles batch b and head group g (4 heads,
256 output features). Pure SPMD, no collectives; host gathers the output.

Per-core device program (all matmul operands fp16; fp32 PSUM accumulation):
  phase 1 (kv stream is only 3.25 MiB, so it goes first while the larger
  q stream arrives behind it on the same HWDGE ring):
    kv [CAP, 512]  = xkv [Wk_g^T|Wv_g^T]  (xkvt: host chunk-transposed)
    M2_p [128,128] = kv_k(pair p)^T @ kv_v(pair p), interleaved one key
                     chunk behind the kv matmuls so they ride the same
                     PE stream (diag 64x64 blocks are M_h; cross terms
                     are zeroed when copied to SBUF)
  phase 2, per 512-query strip (xt is sbk-major so each strip only
  needs its own piece; output DMAs spread across the phase):
    qT strip = Wq_g hs^T ; ctxT strip = blockdiag(M)^T @ qT -> OUT fp16
A few data-independent warmup matmuls run during the DMA head to flip
the PE HAM clock gate to 8/8 before the real work arrives.

Measured end-to-end rel err ~7e-4.
"""
import numpy as np
import concourse.mybir as mybir
import concourse.tile as tile
from concourse import bacc
from concourse.bass_utils import run_bass_kernel_spmd

f32 = mybir.dt.float32
f32r = mybir.dt.float32r
fp16 = mybir.dt.float16

B = 2
S = 2048
D = 1024
DL = 256          # output features per core (4 heads x 64)
KC = D // 128     # 8 contraction chunks
MC = DL // 128    # 2 feature chunks (head pairs)
NSB = S // 512    # 4 query strips
N_CORES = 8
CAP = 1152        # compacted key slots (valid ~Binom(2048,.5): mean 1024,
                  # sd 22.6; 1152 is ~5.7 sigma; fallback covers more)

_cache = {}


def _build(compact, with_bias):
    skv = (CAP if compact else S) // 128   # key chunks
    use_kvm = (not compact) or with_bias   # per-key scale on kv drains
    nc = bacc.Bacc("TRN2", target_bir_lowering=False, debug=False,
                   num_devices=N_CORES)
    XT = nc.declare_dram_parameter("xt", [128, KC * S], fp16, isOutput=False)
    XKVT = nc.declare_dram_parameter("xkvt", [128, skv * D], fp16,
                                     isOutput=False)
    WQT = nc.declare_dram_parameter("wqt", [128, KC * DL], fp16,
                                    isOutput=False)
    WKV = nc.declare_dram_parameter("wkv", [128, KC * 2 * DL], fp16,
                                    isOutput=False)
    if with_bias:
        BQ2 = nc.declare_dram_parameter("bq2", [128, MC], f32, isOutput=False)
        BKV = nc.declare_dram_parameter("bkv", [1, 2 * DL], fp16,
                                        isOutput=False)
        ONE = nc.declare_dram_parameter("ones1", [1, 128], fp16,
                                        isOutput=False)
    if use_kvm:
        KVM = nc.declare_dram_parameter("kvm2", [128, skv], f32,
                                        isOutput=False)
    OUT = nc.declare_dram_parameter("out", [DL, S], fp16, isOutput=True)

    with tile.TileContext(nc) as tc:
        with tc.tile_pool(name="sb", bufs=1) as sb, \
             tc.tile_pool(name="stg", bufs=4) as stg:

            wq_sb = sb.tile([128, KC * DL], fp16, tag="wq")
            xt_sb = sb.tile([128, KC * S], fp16, tag="xt")
            wkv_sb = sb.tile([128, KC * 2 * DL], fp16, tag="wkv")
            xkvt_sb = sb.tile([128, skv * D], fp16, tag="xkvt")
            qT = [sb.tile([128, S], fp16, tag=f"qT{p}", name=f"qT{p}")
                  for p in range(MC)]
            kv_sb = sb.tile([128, skv * 2 * DL], fp16, tag="kv")
            m2s = [sb.tile([128, 128], fp16, tag=f"m2s{p}", name=f"m2s{p}")
                   for p in range(MC)]
            if with_bias:
                bq2 = sb.tile([128, MC], f32, tag="bq2")
                bkv = sb.tile([1, 2 * DL], fp16, tag="bkv")
                ones1 = sb.tile([1, 128], fp16, tag="ones1")
            if use_kvm:
                kvm = sb.tile([128, skv], f32, tag="kvm")

            # DMA order matches PE consumption: kv stream first (smaller,
            # feeds the first phase), then the q stream which has the whole
            # kv phase to arrive. Single HWDGE ring -> FIFO at full BW.
            if with_bias:
                nc.sync.dma_start(bq2[:], BQ2[:, :])
                nc.sync.dma_start(bkv[:], BKV[:, :])
                nc.sync.dma_start(ones1[:], ONE[:, :])
            if use_kvm:
                nc.sync.dma_start(kvm[:], KVM[:, :])
            nc.sync.dma_start(wkv_sb[:], WKV[:, :])
            cuts = [0, D, 3 * D, 6 * D, skv * D]
            for a, b in zip(cuts, cuts[1:]):
                nc.sync.dma_start(xkvt_sb[:, a:b], XKVT[:, a:b])
            nc.sync.dma_start(wq_sb[:], WQT[:, :])
            for i in range(2):
                nc.sync.dma_start(xt_sb[:, i * 8192:(i + 1) * 8192],
                                  XT[:, i * 8192:(i + 1) * 8192])

            # PE warmup: data-independent matmuls on an uninitialized tile
            # run during the DMA head; ~3.5us of activity flips the HAM
            # clock gate to 8/8 so the real matmuls start at 2.4 GHz.
            junk = sb.tile([128, 512], fp16, tag="junk")
            nc.gpsimd.memset(junk[:], 0.0)
            with tc.tile_pool(name="psw", bufs=1, space="PSUM") as psw:
                wps = psw.tile([128, 512], f32, tag="warm")
                for _ in range(12):
                    nc.tensor.matmul(wps[:], junk[:, 0:128], junk[:],
                                     start=True, stop=True)

            # ---- K/V projection over (compacted) keys, with the M-pair
            # matmuls (M2_p = k_p^T v_p) interleaved one chunk behind so
            # they ride the kv matmul stream instead of a separate phase.
            # The psm pool stays open into phase 2: the last M matmuls and
            # the m2s copies slot between the first q groups, so the PE
            # never waits on the final kv drain.
            with tc.tile_pool(name="psm", bufs=1, space="PSUM") as psm, \
                 tc.tile_pool(name="psq", bufs=2, space="PSUM") as psq:
                mps = [psm.tile([128, 128], f32, tag=f"m2_{p}",
                                name=f"m2_{p}") for p in range(MC)]
                # claim a fresh PSUM bank for the first q group NOW, so it
                # isn't handed a just-freed kv bank whose drain is still in
                # flight at the phase transition
                qp_first = psq.tile([128, 512], f32, tag="qp", name="qp_first")

                def m_mms(j):
                    for p in range(MC):
                        nc.tensor.matmul(
                            mps[p][:],
                            kv_sb[:, j * 512 + p * 128:j * 512 + (p + 1) * 128],
                            kv_sb[:, j * 512 + 256 + p * 128:
                                  j * 512 + 256 + (p + 1) * 128],
                            start=(j == 0), stop=(j == skv - 1))

                with tc.tile_pool(name="psk", bufs=3, space="PSUM") as psk:
                    for j in range(skv):
                        pv = psk.tile([128, 512], f32, tag="kv")
                        if with_bias:
                            nc.tensor.matmul(pv[:], ones1[:], bkv[:],
                                             start=True, stop=False)
                        for kc in range(KC):
                            nc.tensor.matmul(
                                pv[:],
                                xkvt_sb[:, j * D + kc * 128:
                                        j * D + (kc + 1) * 128],
                                wkv_sb[:, kc * 512:(kc + 1) * 512],
                                start=(kc == 0 and not with_bias),
                                stop=(kc == KC - 1))
                        if use_kvm:
                            nc.vector.tensor_scalar_mul(
                                kv_sb[:, j * 512:(j + 1) * 512], pv[:],
                                kvm[:, j:j + 1])
                        else:
                            nc.vector.tensor_copy(
                                kv_sb[:, j * 512:(j + 1) * 512], pv[:])
                        if j > 0:
                            m_mms(j - 1)

                # ---- Q projection + M apply, strip by strip --------------
                # xt is laid out sbk-major on the host, so strip sbk only
                # needs its own 1 MiB piece; kc is the inner (accumulation)
                # loop. The ctx matmul and output DMA trail each strip,
                # spreading the output over the whole phase.
                with tc.tile_pool(name="psc", bufs=4, space="PSUM") as psc:

                    def q_group(sbk, mc, qp=None):
                        if qp is None:
                            qp = psq.tile([128, 512], f32, tag="qp")
                        for kc in range(KC):
                            nc.tensor.matmul(
                                qp[:],
                                wq_sb[:, kc * DL + mc * 128:
                                      kc * DL + mc * 128 + 128],
                                xt_sb[:, sbk * (KC * 512) + kc * 512:
                                      sbk * (KC * 512) + (kc + 1) * 512],
                                start=(kc == 0), stop=(kc == KC - 1))
                        if with_bias:
                            nc.vector.tensor_scalar_add(
                                qT[mc][:, sbk * 512:(sbk + 1) * 512], qp[:],
                                bq2[:, mc:mc + 1])
                        else:
                            nc.vector.tensor_copy(
                                qT[mc][:, sbk * 512:(sbk + 1) * 512], qp[:])

                    def ctx_out(sbk, p):
                        cp = psc.tile([128, 512], f32, tag="ctx")
                        nc.tensor.matmul(cp[:], m2s[p][:],
                                         qT[p][:, sbk * 512:(sbk + 1) * 512],
                                         start=True, stop=True)
                        st = stg.tile([128, 512], fp16, tag="st")
                        nc.scalar.copy(st[:], cp[:])
                        out_eng = nc.scalar if p == 0 else nc.sync
                        out_eng.dma_start(
                            OUT[p * 128:(p + 1) * 128,
                                sbk * 512:(sbk + 1) * 512], st[:])

                    for sbk in range(NSB):
                        for mc in range(MC):
                            q_group(sbk, mc,
                                    qp_first if sbk == 0 and mc == 0 else None)
                            if sbk == 0 and mc == 0:
                                # last M chunk + diag extraction ride here,
                                # hidden behind the first q group's drain
                                m_mms(skv - 1)
                                for p in range(MC):
                                    mp = mps[p]
                                    nc.vector.tensor_copy(
                                        m2s[p][0:64, 0:64], mp[0:64, 0:64])
                                    nc.scalar.copy(m2s[p][64:128, 64:128],
                                                   mp[64:128, 64:128])
                                    nc.vector.tensor_scalar_mul(
                                        m2s[p][0:64, 64:128],
                                        mp[0:64, 64:128], 0.0)
                                    nc.vector.tensor_scalar_mul(
                                        m2s[p][64:128, 0:64],
                                        mp[64:128, 0:64], 0.0)
                        for p in range(MC):
                            ctx_out(sbk, p)

    nc.compile()
    return nc


def _get_nc(compact, with_bias):
    key = (compact, with_bias)
    if key not in _cache:
        _cache[key] = _build(compact, with_bias)
    return _cache[key]


def _chunkT(a):
    """[R, D] row-major -> [128, (D//128)*R]: out[p, kc*R + r] = a[r, kc*128+p]."""
    R, Din = a.shape
    return np.ascontiguousarray(
        a.T.reshape(Din // 128, 128, R).transpose(1, 0, 2).reshape(128, -1))


def _make_in_maps(hidden_states, attention_mask, Wq, bq, Wk, bk, Wv, bv):
    hs16 = np.asarray(hidden_states, dtype=np.float32).astype(np.float16)
    am = np.asarray(attention_mask, dtype=np.float32)
    bq = np.asarray(bq, np.float32)
    bk = np.asarray(bk, np.float32)
    bv = np.asarray(bv, np.float32)
    Wq = np.asarray(Wq, np.float32)
    Wk = np.asarray(Wk, np.float32)
    Wv = np.asarray(Wv, np.float32)
    with_bias = bool(bq.any() or bk.any() or bv.any())

    compact = True
    xkvts, kvms = [], []
    for b in range(B):
        valid = np.nonzero(am[b, 0, 0, :] >= 0)[0]
        if len(valid) > CAP:
            compact = False
            break
        xkv = np.zeros((CAP, D), np.float16)
        xkv[:len(valid)] = hs16[b][valid]
        xkvts.append(xkv)
        kvmv = np.zeros(CAP, np.float32)
        kvmv[:len(valid)] = 1.0
        kvms.append(kvmv)

    skv = (CAP if compact else S) // 128
    use_kvm = (not compact) or with_bias

    # xt sbk-major: [p, sbk*4096 + kc*512 + s] = x[sbk*512+s, kc*128+p]
    xts = [np.ascontiguousarray(
        hs16[b].T.reshape(KC, 128, NSB, 512).transpose(1, 2, 0, 3)
        .reshape(128, KC * S)) for b in range(B)]
    if compact:
        xkvt_blk = []
        for b in range(B):
            xkv = xkvts[b]
            blocks = [_chunkT(xkv[j * 128:(j + 1) * 128]) for j in range(skv)]
            xkvt_blk.append(np.ascontiguousarray(np.concatenate(blocks, 1)))
    else:
        xkvt_blk = []
        for b in range(B):
            blocks = [_chunkT(hs16[b][j * 128:(j + 1) * 128])
                      for j in range(skv)]
            xkvt_blk.append(np.ascontiguousarray(np.concatenate(blocks, 1)))

    in_maps = []
    for c in range(N_CORES):
        b, g = divmod(c, 4)
        sl = slice(g * DL, (g + 1) * DL)
        wq_sel = Wq[sl].astype(np.float16)            # [256, 1024]
        wkv_sel = np.vstack([Wk[sl], Wv[sl]]).astype(np.float16)  # [512, 1024]
        m = {
            "xt": xts[b],
            "xkvt": xkvt_blk[b],
            # [p, kc*DL + f] = W[f, kc*128+p]
            "wqt": np.ascontiguousarray(
                wq_sel.T.reshape(KC, 128, DL).transpose(1, 0, 2)
                .reshape(128, KC * DL)),
            "wkv": np.ascontiguousarray(
                wkv_sel.T.reshape(KC, 128, 2 * DL).transpose(1, 0, 2)
                .reshape(128, KC * 2 * DL)),
        }
        if with_bias:
            m["bq2"] = np.ascontiguousarray(
                bq[sl].reshape(MC, 128).T.astype(np.float32))
            m["bkv"] = np.ascontiguousarray(
                np.concatenate([bk[sl], bv[sl]]).reshape(1, 2 * DL)
                .astype(np.float16))
            m["ones1"] = np.ones((1, 128), np.float16)
        if use_kvm:
            if compact:
                kvmv = kvms[b]
            else:
                kvmv = (am[b, 0, 0, :] >= 0).astype(np.float32)
            m["kvm2"] = np.ascontiguousarray(kvmv.reshape(skv, 128).T)
        in_maps.append(m)
    return (compact, with_bias), in_maps


def _gather(results):
    out = np.empty((B, S, D), np.float32)
    for c in range(N_CORES):
        b, g = divmod(c, 4)
        out[b, :, g * DL:(g + 1) * DL] = results[c]["out"].T.astype(np.float32)
    return out


def run_sharded(variant, in_maps, **kw):
    compact, with_bias = variant if isinstance(variant, tuple) else (variant, False)
    nc = _get_nc(compact, with_bias)
    return run_bass_kernel_spmd(nc, in_maps, core_ids=list(range(N_CORES)), **kw)


def kernel(hidden_states, attention_mask, Wq, bq, Wk, bk, Wv, bv):
    variant, in_maps = _make_in_maps(hidden_states, attention_mask,
                                     Wq, bq, Wk, bk, Wv, bv)
    res = run_sharded(variant, in_maps)
    return _gather(res.results)
